# revision 2
# baseline (speedup 1.0000x reference)
"""Trainium2 Bass kernel v2 for nn_CombinedHiddenEncoder.

Same algebraic restructure as v1 (S = D^-1/2 (A+I) D^-1/2 shared by all five
GCNConvs commutes with the weight matrices):

    U      = feature @ (W1 @ W3[:HD]) + condition @ (W2 @ W3[HD:])
    X0     = T U                      (T = diag(1/sqrt(deg)))
    X1     = T^2 A X0,  X2 = T^2 A X1,     (A = 0/1 adjacency incl. I)
    rawV   = A X2      (V = T rawV)
    mean   = dinv * (rawV @ Wm + sr' @ Cm)      sr' = [s2/di, s1/di, 1/di]
    logvar = dinv * (rawV @ Wv + sr' @ Cv)
    z      = noise * exp(0.5*logvar) + mean

v2 changes vs v1:
  * All sparse-path data (table, gather, one-hot, strips) in bf16: halves
    gather + AllGather bytes and doubles PE matmul rate.  PSUM stays fp32.
  * Tight slot packing: per-(dst-tile, src-parity) segment capacity CAP =
    actual max count over all (core, tile, parity) groups (~870) instead of
    a 128-multiple (1024): ~15% fewer gather slots.  Self loops are not
    edges by default; they fill slack slots (up to 64/parity/tile), and the
    leftovers are applied via one masked-diagonal matmul per tile whose rhs
    is the core-local strip (no gather at all for them).
  * Final round accumulates the TRANSPOSED psum (swap matmul operands), so
    the head needs no PE transpose; the final dinv scaling moves onto the
    small [128, LD] head psums (per-partition scalar).
  * Head noise/z/mean/logvar are strips with one rearranged DMA each.
"""

import numpy as np

import concourse.bass as bass
import concourse.mybir as mybir
import concourse.tile as tile
from concourse import bacc
from concourse.bass_utils import run_bass_kernel_spmd
from concourse.masks import make_identity

F32 = mybir.dt.float32
BF16 = mybir.dt.bfloat16
I16 = mybir.dt.int16

N, E = 50000, 600000
FD, CD, HD, LD = 256, 128, 128, 64
CORES = 8
SHARD = N // CORES            # 6250
TILES = (SHARD + 127) // 128  # 49
R = TILES * 128               # 6272 padded rows per core
TR = CORES * R                # 50176 table rows
TPG = 7                       # dst-tiles per gather call group
NGROUPS = TILES // TPG        # 7

_prog_cache: dict = {}


def _mm_descs(cap):
    """Static matmul descriptor list: (par, chunk, tile, mcol), tile-major
    with parity inner, chunks ascending.  Shared by host and builder."""
    descs = []
    mcol = 0
    for t in range(TILES):
        for p in (0, 1):
            c0 = (t * cap) // 128
            c1 = ((t + 1) * cap - 1) // 128
            for c in range(c0, c1 + 1):
                descs.append((p, c, t, mcol))
                mcol += 1
    return descs, mcol


def _call_bounds(cap):
    """Gather call chunk ranges per parity: call g covers the chunks needed
    by tiles [7g, 7g+7) that were not gathered by earlier calls."""
    nchk = (TILES * cap + 127) // 128
    bounds = []
    s = 0
    for g in range(NGROUPS):
        e = nchk if g == NGROUPS - 1 else ((TPG * (g + 1)) * cap - 1) // 128 + 1
        bounds.append((s, e))
        s = e
    return nchk, bounds


# --------------------------------------------------------------------------
# Bass program builder
# --------------------------------------------------------------------------
def build_program(cap: int, variant: str = "full"):
    do_gather = variant != "nogather"
    do_mm = variant != "nomm"
    do_cc = variant != "nocc"
    do_q = variant != "noq"
    nc = bacc.Bacc(None, target_bir_lowering=False)

    descs, nmm = _mm_descs(cap)
    nchk, bounds = _call_bounds(cap)
    # per-parity chunk -> (call index, offset within call)
    call_of_chunk = {}
    for g, (s, e) in enumerate(bounds):
        for c in range(s, e):
            call_of_chunk[c] = (g, c - s)
    idxcols = sum((e - s) * 8 for s, e in bounds)  # per parity

    # ---- I/O ----
    xfT = nc.dram_tensor("xfT", [FD, R], BF16, kind="ExternalInput")
    xcT = nc.dram_tensor("xcT", [CD, R], BF16, kind="ExternalInput")
    noise_in = nc.dram_tensor("noise_in", [R, LD], F32, kind="ExternalInput")
    aw = nc.dram_tensor("aw", [FD, HD], BF16, kind="ExternalInput")
    bw = nc.dram_tensor("bw", [CD, HD], BF16, kind="ExternalInput")
    wm = nc.dram_tensor("wm", [HD, LD], BF16, kind="ExternalInput")
    wv = nc.dram_tensor("wv", [HD, LD], BF16, kind="ExternalInput")
    cm = nc.dram_tensor("cm", [4, LD], BF16, kind="ExternalInput")
    cv = nc.dram_tensor("cv", [4, LD], BF16, kind="ExternalInput")
    dinv_p = nc.dram_tensor("dinv_p", [128, TILES], F32, kind="ExternalInput")
    dinv2_p = nc.dram_tensor("dinv2_p", [128, TILES], F32, kind="ExternalInput")
    dinvh_p = nc.dram_tensor("dinvh_p", [128, TILES], F32, kind="ExternalInput")
    dmask_p = nc.dram_tensor("dmask_p", [128, TILES], F32, kind="ExternalInput")
    srows = nc.dram_tensor("srows", [4, R], BF16, kind="ExternalInput")
    idx_all = nc.dram_tensor("idx_all", [128, 2 * idxcols], I16,
                             kind="ExternalInput")
    dloc_all = nc.dram_tensor("dloc_all", [128, nmm], F32,
                              kind="ExternalInput")

    z_out = nc.dram_tensor("z_out", [R, LD], F32, kind="ExternalOutput")
    mean_out = nc.dram_tensor("mean_out", [R, LD], F32, kind="ExternalOutput")
    logvar_out = nc.dram_tensor("logvar_out", [R, LD], F32,
                                kind="ExternalOutput")

    # ---- internal DRAM ----
    bounce = [nc.dram_tensor(f"xb{r}", [R, HD], BF16) for r in range(3)]
    tabs = [nc.dram_tensor(f"tab{r}", [TR, HD], BF16, addr_space="Shared")
            for r in range(3)]
    rg = [list(range(CORES))]

    with tile.TileContext(nc) as tc:
        with tc.tile_pool(name="const", bufs=1) as cpool:
            ident = cpool.tile([128, 128], BF16)
            make_identity(nc, ident[:])
            colidx = cpool.tile([128, 128], BF16)
            nc.gpsimd.iota(colidx[:], pattern=[[1, 128]], base=0,
                           channel_multiplier=0,
                           allow_small_or_imprecise_dtypes=True)
            a0_s = cpool.tile([128, HD], BF16)
            a1_s = cpool.tile([128, HD], BF16)
            b_s = cpool.tile([128, HD], BF16)
            nc.sync.dma_start(out=a0_s[:], in_=aw[0:128, :])
            nc.sync.dma_start(out=a1_s[:], in_=aw[128:256, :])
            nc.sync.dma_start(out=b_s[:], in_=bw[:, :])
            wm_s = cpool.tile([128, LD], BF16)
            wv_s = cpool.tile([128, LD], BF16)
            nc.sync.dma_start(out=wm_s[:], in_=wm[:, :])
            nc.sync.dma_start(out=wv_s[:], in_=wv[:, :])
            cm_s = cpool.tile([4, LD], BF16)
            cv_s = cpool.tile([4, LD], BF16)
            nc.sync.dma_start(out=cm_s[:4, :], in_=cm[:, :])
            nc.sync.dma_start(out=cv_s[:4, :], in_=cv[:, :])
            dinv_s = cpool.tile([128, TILES], F32)
            dinv2_s = cpool.tile([128, TILES], F32)
            dinvh_s = cpool.tile([128, TILES], F32)
            dmask_s = cpool.tile([128, TILES], F32)
            nc.sync.dma_start(out=dinv_s[:], in_=dinv_p[:, :])
            nc.sync.dma_start(out=dinv2_s[:], in_=dinv2_p[:, :])
            nc.sync.dma_start(out=dinvh_s[:], in_=dinvh_p[:, :])
            nc.sync.dma_start(out=dmask_s[:], in_=dmask_p[:, :])
            sr_s = cpool.tile([4, R], BF16)
            nc.sync.dma_start(out=sr_s[:4, :], in_=srows[:, :])
            idx_s = cpool.tile([128, 2 * idxcols], I16)
            nc.sync.dma_start(out=idx_s[:], in_=idx_all[:, :])
            dloc_s = cpool.tile([128, nmm], F32)
            nc.sync.dma_start(out=dloc_s[:], in_=dloc_all[:, :])
            nzstrip = cpool.tile([128, TILES * LD], F32)
            nc.sync.dma_start(
                out=nzstrip[:].rearrange("p (t h) -> p t h", h=LD),
                in_=noise_in.ap().rearrange("(t p) h -> p t h", p=128))
            # per-tile 0/1 diagonal leftover-self-loop masks (constant
            # across rounds): dg[t] = ident * dmask[:, t]
            dgs = cpool.tile([128, TILES * 128], BF16)
            for t in range(TILES):
                nc.vector.tensor_scalar(
                    out=dgs[:, t * 128:(t + 1) * 128], in0=ident[:],
                    scalar1=dmask_s[:, t:t + 1], scalar2=None,
                    op0=mybir.AluOpType.mult)

            with tc.tile_pool(name="psum", bufs=4, space="PSUM") as mmpool, \
                 tc.tile_pool(name="hpsum", bufs=4, space="PSUM") as hpool:

                # ---------------- stage 0: X0 = T (Xf A + Xc B) --------------
                with tc.tile_pool(name="strips", bufs=1) as spool:
                    xstrip = [spool.tile([128, R], BF16, name=f"xs{i}")
                              for i in range(2)]
                    with tc.tile_pool(name="s0", bufs=1) as s0pool:
                        xf0_s = s0pool.tile([128, R], BF16)
                        xf1_s = s0pool.tile([128, R], BF16)
                        xc_s = s0pool.tile([128, R], BF16)
                        nc.sync.dma_start(out=xf0_s[:], in_=xfT[0:128, :])
                        nc.sync.dma_start(out=xf1_s[:], in_=xfT[128:256, :])
                        nc.sync.dma_start(out=xc_s[:], in_=xcT[:, :])
                        for t in range(TILES):
                            cs = slice(t * 128, (t + 1) * 128)
                            ps = mmpool.tile([128, HD], F32, name=f"s0ps{t}",
                                             tag="mm")
                            nc.tensor.matmul(ps[:], xf0_s[:, cs], a0_s[:],
                                             start=True, stop=False)
                            nc.tensor.matmul(ps[:], xf1_s[:, cs], a1_s[:],
                                             start=False, stop=False)
                            nc.tensor.matmul(ps[:], xc_s[:, cs], b_s[:],
                                             start=False, stop=True)
                            nc.vector.tensor_scalar(
                                out=xstrip[0][:, cs], in0=ps[:],
                                scalar1=dinv_s[:, t:t + 1], scalar2=None,
                                op0=mybir.AluOpType.mult)
                        nc.sync.dma_start(
                            out=bounce[0].ap().rearrange("(t p) h -> p t h",
                                                         p=128),
                            in_=xstrip[0][:].rearrange("p (t h) -> p t h",
                                                       h=HD))

                        if do_cc:
                            nc.gpsimd.collective_compute(
                                "AllGather", mybir.AluOpType.bypass,
                                replica_groups=rg,
                                ins=[bounce[0].ap()], outs=[tabs[0].ap()])

                    # ---------------- 3 sparse rounds ------------------------
                    with tc.tile_pool(name="gpool", bufs=4) as gpool, \
                         tc.tile_pool(name="qpool", bufs=8) as qpool, \
                         tc.tile_pool(name="hd", bufs=4) as hdpool, \
                         tc.tile_pool(name="ostr", bufs=1) as opool:
                        znstrip = opool.tile([128, TILES * LD], F32)
                        mnstrip = opool.tile([128, TILES * LD], F32)
                        lvstrip = opool.tile([128, TILES * LD], F32)
                        for rnd in range(3):
                            xprev = xstrip[rnd % 2]
                            xcur = xstrip[(rnd + 1) % 2]
                            tab = tabs[rnd]
                            tab2 = tab.ap().rearrange(
                                "(r two) h -> r two h", two=2)
                            last = rnd == 2
                            gts = {}   # (par, call) -> gather tile
                            for g in range(NGROUPS):
                                s, e = bounds[g]
                                ncall = e - s
                                for par in (0, 1):
                                    gt = gpool.tile(
                                        [128, ncall, 128], BF16,
                                        name=f"gt{rnd}_{g}_{par}", tag="gath")
                                    ic0 = par * idxcols + \
                                        sum((bounds[j][1] - bounds[j][0]) * 8
                                            for j in range(g))
                                    if do_gather:
                                        nc.gpsimd.dma_gather(
                                            out_ap=gt[:],
                                            in_ap=tab2[:, par, :],
                                            idxs_ap=idx_s[:,
                                                          ic0:ic0 + ncall * 8],
                                            num_idxs=ncall * 128,
                                            num_idxs_reg=ncall * 128,
                                            elem_size=HD,
                                            elem_step=2 * HD,
                                            single_packet=False)
                                    else:
                                        nc.vector.tensor_scalar(
                                            out=gt[:, 0, :], in0=colidx[:],
                                            scalar1=1.0, scalar2=None,
                                            op0=mybir.AluOpType.mult)
                                    gts[(par, g)] = gt
                                for ti in range(TPG):
                                    t = g * TPG + ti
                                    cs = slice(t * 128, (t + 1) * 128)
                                    ps = mmpool.tile([128, HD], F32,
                                                     name=f"ps{rnd}_{t}",
                                                     tag="mm")
                                    tdescs = [d for d in descs if d[2] == t]
                                    if not do_mm:
                                        tdescs = []
                                    qshared = None
                                    for j, (p, c, _, m) in enumerate(tdescs):
                                        cg, coff = call_of_chunk[c]
                                        if do_q or qshared is None:
                                            q = qpool.tile(
                                                [128, 128], BF16,
                                                name=f"q{rnd}_{t}_{j}",
                                                tag="q")
                                            nc.vector.tensor_scalar(
                                                out=q[:], in0=colidx[:],
                                                scalar1=dloc_s[:, m:m + 1],
                                                scalar2=None,
                                                op0=mybir.AluOpType.is_equal)
                                            qshared = q
                                        else:
                                            q = qshared
                                        gsl = gts[(p, cg)][:, coff, :]
                                        if not last:
                                            nc.tensor.matmul(
                                                ps[:], q[:], gsl,
                                                start=(j == 0), stop=False)
                                        else:
                                            nc.tensor.matmul(
                                                ps[:], gsl, q[:],
                                                start=(j == 0), stop=False)
                                    dg = dgs[:, t * 128:(t + 1) * 128]
                                    if not last:
                                        nc.tensor.matmul(
                                            ps[:], dg, xprev[:, cs],
                                            start=not tdescs, stop=True)
                                        nc.vector.tensor_scalar(
                                            out=xcur[:, cs], in0=ps[:],
                                            scalar1=dinv2_s[:, t:t + 1],
                                            scalar2=None,
                                            op0=mybir.AluOpType.mult)
                                    else:
                                        nc.tensor.matmul(
                                            ps[:], xprev[:, cs], dg,
                                            start=not tdescs, stop=True)
                                        # ps is rawV^T [feat, node]
                                        vT = hdpool.tile([128, HD], BF16,
                                                         name=f"vT{t}",
                                                         tag="vT")
                                        nc.vector.tensor_copy(out=vT[:],
                                                              in_=ps[:])
                                        os = slice(t * LD, (t + 1) * LD)
                                        mps = hpool.tile([128, LD], F32,
                                                         name=f"mps{t}",
                                                         tag="hp")
                                        nc.tensor.matmul(mps[:], vT[:],
                                                         wm_s[:],
                                                         start=True,
                                                         stop=False)
                                        nc.tensor.matmul(mps[:],
                                                         sr_s[:3, cs],
                                                         cm_s[:3, :],
                                                         start=False,
                                                         stop=True)
                                        lps = hpool.tile([128, LD], F32,
                                                         name=f"lps{t}",
                                                         tag="hp")
                                        nc.tensor.matmul(lps[:], vT[:],
                                                         wv_s[:],
                                                         start=True,
                                                         stop=False)
                                        nc.tensor.matmul(lps[:],
                                                         sr_s[:3, cs],
                                                         cv_s[:3, :],
                                                         start=False,
                                                         stop=True)
                                        nc.vector.tensor_scalar(
                                            out=mnstrip[:, os], in0=mps[:],
                                            scalar1=dinv_s[:, t:t + 1],
                                            scalar2=None,
                                            op0=mybir.AluOpType.mult)
                                        nc.vector.tensor_scalar(
                                            out=lvstrip[:, os], in0=lps[:],
                                            scalar1=dinv_s[:, t:t + 1],
                                            scalar2=None,
                                            op0=mybir.AluOpType.mult)
                                        ex = hdpool.tile([128, LD], F32,
                                                         name=f"ex{t}",
                                                         tag="ex")
                                        nc.scalar.activation(
                                            out=ex[:], in_=lps[:],
                                            func=mybir.ActivationFunctionType.Exp,
                                            scale=dinvh_s[:, t:t + 1])
                                        zt = hdpool.tile([128, LD], F32,
                                                         name=f"zt{t}",
                                                         tag="zt")
                                        nc.vector.tensor_tensor(
                                            out=zt[:], in0=nzstrip[:, os],
                                            in1=ex[:],
                                            op=mybir.AluOpType.mult)
                                        nc.vector.tensor_tensor(
                                            out=znstrip[:, os], in0=zt[:],
                                            in1=mnstrip[:, os],
                                            op=mybir.AluOpType.add)
                            if not last:
                                nc.sync.dma_start(
                                    out=bounce[rnd + 1].ap().rearrange(
                                        "(t p) h -> p t h", p=128),
                                    in_=xcur[:].rearrange(
                                        "p (t h) -> p t h", h=HD))
                                if do_cc:
                                    nc.gpsimd.collective_compute(
                                        "AllGather", mybir.AluOpType.bypass,
                                        replica_groups=rg,
                                        ins=[bounce[rnd + 1].ap()],
                                        outs=[tabs[rnd + 1].ap()])
                        nc.sync.dma_start(
                            out=z_out.ap().rearrange("(t p) h -> p t h",
                                                     p=128),
                            in_=znstrip[:].rearrange("p (t h) -> p t h",
                                                     h=LD))
                        nc.sync.dma_start(
                            out=mean_out.ap().rearrange("(t p) h -> p t h",
                                                        p=128),
                            in_=mnstrip[:].rearrange("p (t h) -> p t h",
                                                     h=LD))
                        nc.sync.dma_start(
                            out=logvar_out.ap().rearrange("(t p) h -> p t h",
                                                          p=128),
                            in_=lvstrip[:].rearrange("p (t h) -> p t h",
                                                     h=LD))
    nc.finalize()
    return nc


# --------------------------------------------------------------------------
# Host-side preprocessing
# --------------------------------------------------------------------------
def preprocess(feature, condition, edge_index, noise,
               W1, b1, W2, b2, W3, b3, Wm, bm, Wv, bv):
    feature = np.asarray(feature, np.float32)
    condition = np.asarray(condition, np.float32)
    noise = np.asarray(noise, np.float32)
    ei = np.asarray(edge_index).astype(np.int64)
    W1 = np.asarray(W1, np.float32); b1 = np.asarray(b1, np.float32)
    W2 = np.asarray(W2, np.float32); b2 = np.asarray(b2, np.float32)
    W3 = np.asarray(W3, np.float32); b3 = np.asarray(b3, np.float32)
    Wm = np.asarray(Wm, np.float32); bm = np.asarray(bm, np.float32)
    Wv = np.asarray(Wv, np.float32); bv = np.asarray(bv, np.float32)

    src, dst = ei[0], ei[1]
    loop = np.arange(N, dtype=np.int64)
    deg = (np.bincount(np.concatenate([dst, loop]), minlength=N)
           .astype(np.float64))
    dinv = 1.0 / np.sqrt(deg)
    asrc = np.concatenate([src, loop])
    adst = np.concatenate([dst, loop])
    w = dinv[asrc] * dinv[adst]
    s1 = np.bincount(adst, weights=w, minlength=N)
    s2 = np.bincount(adst, weights=w * s1[asrc], minlength=N)
    dinv32 = dinv.astype(np.float32)

    W3a, W3b = W3[:HD], W3[HD:]
    A_w = (W1 @ W3a).astype(np.float32)
    B_w = (W2 @ W3b).astype(np.float32)
    c1 = b1 @ W3a + b2 @ W3b
    Cm = np.zeros((4, LD), np.float32)
    Cm[:3] = np.stack([c1 @ Wm, b3 @ Wm, bm])
    Cv = np.zeros((4, LD), np.float32)
    Cv[:3] = np.stack([c1 @ Wv, b3 @ Wv, bv])

    node = np.arange(N, dtype=np.int64)
    pos_of_node = (node // SHARD) * R + (node % SHARD)
    pos_src = pos_of_node[src]
    core = dst // SHARD
    d_loc = dst - core * SHARD
    tl = d_loc // 128
    dloc = d_loc % 128
    par = (pos_src & 1).astype(np.int64)
    idx16 = (pos_src >> 1).astype(np.int64)

    # counts per (core, tile, parity) -> CAP
    gid = (core * TILES + tl) * 2 + par
    counts = np.bincount(gid, minlength=CORES * TILES * 2)
    cap = int(counts.max())

    descs, nmm = _mm_descs(cap)
    nchk, bounds = _call_bounds(cap)
    nslot = nchk * 128

    # order edges by (core, tile, parity), dloc ascending within the group
    order = np.lexsort((dloc, gid))
    gs = gid[order]
    starts = np.concatenate([[0], np.cumsum(counts)[:-1]])
    within = np.arange(len(gs)) - np.repeat(starts, counts)

    ocore = core[order]
    opar = par[order]
    otl = tl[order]
    # slot index within the (core, parity) block
    slot = otl * cap + within

    bf = np.float32  # host arrays later cast

    in_maps = []
    for k in range(CORES):
        rows = slice(k * SHARD, (k + 1) * SHARD)
        idx_p = np.zeros((2, nslot), np.int16)
        dl_p = np.full((2, nslot), -1.0, np.float32)
        fill_n = np.zeros((2, TILES), np.int64)
        for p in (0, 1):
            m = (ocore == k) & (opar == p)
            idx_p[p][slot[m]] = idx16[order][m].astype(np.int16)
            dl_p[p][slot[m]] = dloc[order][m].astype(np.float32)

        # self-loop filler: node i (tile t, row parity q) contributes X[i]
        # to psum[i]; append into slack slots of segment (t, q).
        dmask = np.zeros((TILES, 128), np.float32)
        cnt_k = counts.reshape(CORES, TILES, 2)[k]
        for t in range(TILES):
            nreal = min(128, SHARD - t * 128)
            i_local = t * 128 + np.arange(nreal)
            pos_i = k * R + i_local
            q_i = pos_i & 1
            for p in (0, 1):
                cand = i_local[q_i == p]
                space = cap - cnt_k[t, p]
                use = cand[:space]
                rest = cand[space:]
                base = t * cap + cnt_k[t, p]
                idx_p[p][base:base + len(use)] = (
                    (k * R + use) >> 1).astype(np.int16)
                dl_p[p][base:base + len(use)] = (use - t * 128).astype(
                    np.float32)
                dmask[t][rest - t * 128] = 1.0

        # wrap idx lists per call: [16, n/16] replicated to 128 partitions
        idx_cols = []
        for p in (0, 1):
            for s, e in bounds:
                iv = idx_p[p][s * 128:e * 128]
                ic = iv.reshape(-1, 16).T          # [16, n/16]
                idx_cols.append(ic)
        # order in SBUF: parity-major (par * idxcols + call offset)
        ic_all = np.concatenate(idx_cols, axis=1)
        idx_arr = np.tile(ic_all, (8, 1))

        # dloc columns per matmul descriptor
        dl_arr = np.full((128, nmm), -1.0, np.float32)
        for p, c, t, mcol in descs:
            seg = dl_p[p][c * 128:(c + 1) * 128]
            # the chunk may contain other tiles' edges -> keep -1 there
            s0, s1_ = t * cap, (t + 1) * cap
            pos = np.arange(c * 128, (c + 1) * 128)
            ok = (pos >= s0) & (pos < s1_) & (seg >= 0)
            col = np.where(ok, seg, -1.0)
            dl_arr[:, mcol] = col

        xfTb = np.zeros((FD, R), bf)
        xfTb[:, :SHARD] = feature[rows].T
        xcTb = np.zeros((CD, R), bf)
        xcTb[:, :SHARD] = condition[rows].T
        nz = np.zeros((R, LD), np.float32)
        nz[:SHARD] = noise[rows]
        dv = np.zeros((TILES, 128), np.float32)
        dv.reshape(-1)[:SHARD] = dinv32[rows]
        sr = np.zeros((4, R), np.float32)
        di = dinv[rows]
        sr[0, :SHARD] = (s2[rows] / di).astype(np.float32)
        sr[1, :SHARD] = (s1[rows] / di).astype(np.float32)
        sr[2, :SHARD] = (1.0 / di).astype(np.float32)

        in_maps.append({
            "xfT": xfTb, "xcT": xcTb, "noise_in": nz,
            "aw": A_w, "bw": B_w, "wm": Wm, "wv": Wv, "cm": Cm, "cv": Cv,
            "dinv_p": np.ascontiguousarray(dv.T),
            "dinv2_p": np.ascontiguousarray((dv ** 2).T),
            "dinvh_p": np.ascontiguousarray(0.5 * dv.T),
            "dmask_p": np.ascontiguousarray(dmask.T),
            "srows": sr,
            "idx_all": np.ascontiguousarray(idx_arr),
            "dloc_all": np.ascontiguousarray(dl_arr),
        })
    return cap, in_maps


def _cast_maps(in_maps, nc):
    """Cast host fp32 arrays to the program's declared dtypes (bf16)."""
    import ml_dtypes
    bf16_names = {"xfT", "xcT", "aw", "bw", "wm", "wv", "cm", "cv",
                  "srows"}
    out = []
    for m in in_maps:
        d = dict(m)
        for n in bf16_names:
            d[n] = m[n].astype(ml_dtypes.bfloat16)
        out.append(d)
    return out


def kernel(feature, condition, edge_index, noise,
           W1, b1, W2, b2, W3, b3, Wm, bm, Wv, bv, _trace=False):
    cap, in_maps = preprocess(feature, condition, edge_index, noise,
                              W1, b1, W2, b2, W3, b3, Wm, bm, Wv, bv)
    if cap not in _prog_cache:
        _prog_cache[cap] = build_program(cap)
    nc = _prog_cache[cap]
    in_maps = _cast_maps(in_maps, nc)
    res = run_bass_kernel_spmd(nc, in_maps, list(range(CORES)), trace=_trace)
    z = np.concatenate([res.results[k]["z_out"][:SHARD] for k in range(CORES)])
    mean = np.concatenate(
        [res.results[k]["mean_out"][:SHARD] for k in range(CORES)])
    logvar = np.concatenate(
        [res.results[k]["logvar_out"][:SHARD] for k in range(CORES)])
    return (z, mean, logvar)


# revision 3
# speedup vs baseline: 1.4194x; 1.4194x over previous
"""Trainium2 Bass kernel v2 for nn_CombinedHiddenEncoder.

Same algebraic restructure as v1 (S = D^-1/2 (A+I) D^-1/2 shared by all five
GCNConvs commutes with the weight matrices):

    U      = feature @ (W1 @ W3[:HD]) + condition @ (W2 @ W3[HD:])
    X0     = T U                      (T = diag(1/sqrt(deg)))
    X1     = T^2 A X0,  X2 = T^2 A X1,     (A = 0/1 adjacency incl. I)
    rawV   = A X2      (V = T rawV)
    mean   = dinv * (rawV @ Wm + sr' @ Cm)      sr' = [s2/di, s1/di, 1/di]
    logvar = dinv * (rawV @ Wv + sr' @ Cv)
    z      = noise * exp(0.5*logvar) + mean

v2 changes vs v1:
  * All sparse-path data (table, gather, one-hot, strips) in bf16: halves
    gather + AllGather bytes and doubles PE matmul rate.  PSUM stays fp32.
  * Tight slot packing: per-(dst-tile, src-parity) segment capacity CAP =
    actual max count over all (core, tile, parity) groups (~870) instead of
    a 128-multiple (1024): ~15% fewer gather slots.  Self loops are not
    edges by default; they fill slack slots (up to 64/parity/tile), and the
    leftovers are applied via one masked-diagonal matmul per tile whose rhs
    is the core-local strip (no gather at all for them).
  * Final round accumulates the TRANSPOSED psum (swap matmul operands), so
    the head needs no PE transpose; the final dinv scaling moves onto the
    small [128, LD] head psums (per-partition scalar).
  * Head noise/z/mean/logvar are strips with one rearranged DMA each.
"""

import numpy as np

import concourse.bass as bass
import concourse.mybir as mybir
import concourse.tile as tile
from concourse import bacc
from concourse.bass_utils import run_bass_kernel_spmd
from concourse.masks import make_identity

F32 = mybir.dt.float32
BF16 = mybir.dt.bfloat16
I16 = mybir.dt.int16

N, E = 50000, 600000
FD, CD, HD, LD = 256, 128, 128, 64
CORES = 8
SHARD = N // CORES            # 6250
TILES = (SHARD + 127) // 128  # 49
R = TILES * 128               # 6272 padded rows per core
TR = CORES * R                # 50176 table rows
TPG = 7                       # dst-tiles per gather call group
NGROUPS = TILES // TPG        # 7

_prog_cache: dict = {}


def _mm_descs(cap):
    """Static matmul descriptor list: (par, chunk, tile, mcol), tile-major
    with parity inner, chunks ascending.  Shared by host and builder."""
    descs = []
    mcol = 0
    for t in range(TILES):
        for p in (0, 1):
            c0 = (t * cap) // 128
            c1 = ((t + 1) * cap - 1) // 128
            for c in range(c0, c1 + 1):
                descs.append((p, c, t, mcol))
                mcol += 1
    return descs, mcol


def _call_bounds(cap):
    """Gather call chunk ranges per parity: call g covers the chunks needed
    by tiles [7g, 7g+7) that were not gathered by earlier calls."""
    nchk = (TILES * cap + 127) // 128
    bounds = []
    s = 0
    for g in range(NGROUPS):
        e = nchk if g == NGROUPS - 1 else ((TPG * (g + 1)) * cap - 1) // 128 + 1
        bounds.append((s, e))
        s = e
    return nchk, bounds


# --------------------------------------------------------------------------
# Bass program builder
# --------------------------------------------------------------------------
def build_program(cap: int, variant: str = "full"):
    do_gather = variant != "nogather"
    do_mm = variant != "nomm"
    do_cc = variant != "nocc"
    do_q = variant != "noq"
    nc = bacc.Bacc(None, target_bir_lowering=False)

    descs, nmm = _mm_descs(cap)
    nchk, bounds = _call_bounds(cap)
    # per-parity chunk -> (call index, offset within call)
    call_of_chunk = {}
    for g, (s, e) in enumerate(bounds):
        for c in range(s, e):
            call_of_chunk[c] = (g, c - s)
    idxcols = sum((e - s) * 8 for s, e in bounds)  # per parity

    # ---- I/O ----
    xfT = nc.dram_tensor("xfT", [FD, R], BF16, kind="ExternalInput")
    xcT = nc.dram_tensor("xcT", [CD, R], BF16, kind="ExternalInput")
    noise_in = nc.dram_tensor("noise_in", [R, LD], F32, kind="ExternalInput")
    aw = nc.dram_tensor("aw", [FD, HD], BF16, kind="ExternalInput")
    bw = nc.dram_tensor("bw", [CD, HD], BF16, kind="ExternalInput")
    wm = nc.dram_tensor("wm", [HD, LD], BF16, kind="ExternalInput")
    wv = nc.dram_tensor("wv", [HD, LD], BF16, kind="ExternalInput")
    cm = nc.dram_tensor("cm", [4, LD], BF16, kind="ExternalInput")
    cv = nc.dram_tensor("cv", [4, LD], BF16, kind="ExternalInput")
    dinv_p = nc.dram_tensor("dinv_p", [128, TILES], F32, kind="ExternalInput")
    dinv2_p = nc.dram_tensor("dinv2_p", [128, TILES], F32, kind="ExternalInput")
    dinvh_p = nc.dram_tensor("dinvh_p", [128, TILES], F32, kind="ExternalInput")
    dmask_p = nc.dram_tensor("dmask_p", [128, TILES], F32, kind="ExternalInput")
    srows = nc.dram_tensor("srows", [4, R], BF16, kind="ExternalInput")
    idx_all = nc.dram_tensor("idx_all", [128, 2 * idxcols], I16,
                             kind="ExternalInput")
    dloc_all = nc.dram_tensor("dloc_all", [128, nmm], F32,
                              kind="ExternalInput")

    z_out = nc.dram_tensor("z_out", [R, LD], F32, kind="ExternalOutput")
    mean_out = nc.dram_tensor("mean_out", [R, LD], F32, kind="ExternalOutput")
    logvar_out = nc.dram_tensor("logvar_out", [R, LD], F32,
                                kind="ExternalOutput")

    # ---- internal DRAM ----
    bounce = [nc.dram_tensor(f"xb{r}", [R, HD], BF16) for r in range(3)]
    tabs = [nc.dram_tensor(f"tab{r}", [TR, HD], BF16, addr_space="Shared")
            for r in range(3)]
    rg = [list(range(CORES))]

    with tile.TileContext(nc) as tc:
        with tc.tile_pool(name="const", bufs=1) as cpool:
            ident = cpool.tile([128, 128], BF16)
            make_identity(nc, ident[:])
            colidx = cpool.tile([128, 128], BF16)
            nc.gpsimd.iota(colidx[:], pattern=[[1, 128]], base=0,
                           channel_multiplier=0,
                           allow_small_or_imprecise_dtypes=True)
            a0_s = cpool.tile([128, HD], BF16)
            a1_s = cpool.tile([128, HD], BF16)
            b_s = cpool.tile([128, HD], BF16)
            nc.sync.dma_start(out=a0_s[:], in_=aw[0:128, :])
            nc.sync.dma_start(out=a1_s[:], in_=aw[128:256, :])
            nc.sync.dma_start(out=b_s[:], in_=bw[:, :])
            wm_s = cpool.tile([128, LD], BF16)
            wv_s = cpool.tile([128, LD], BF16)
            nc.sync.dma_start(out=wm_s[:], in_=wm[:, :])
            nc.sync.dma_start(out=wv_s[:], in_=wv[:, :])
            cm_s = cpool.tile([4, LD], BF16)
            cv_s = cpool.tile([4, LD], BF16)
            nc.sync.dma_start(out=cm_s[:4, :], in_=cm[:, :])
            nc.sync.dma_start(out=cv_s[:4, :], in_=cv[:, :])
            dinv_s = cpool.tile([128, TILES], F32)
            dinv2_s = cpool.tile([128, TILES], F32)
            dinvh_s = cpool.tile([128, TILES], F32)
            dmask_s = cpool.tile([128, TILES], F32)
            nc.sync.dma_start(out=dinv_s[:], in_=dinv_p[:, :])
            nc.sync.dma_start(out=dinv2_s[:], in_=dinv2_p[:, :])
            nc.sync.dma_start(out=dinvh_s[:], in_=dinvh_p[:, :])
            nc.sync.dma_start(out=dmask_s[:], in_=dmask_p[:, :])
            sr_s = cpool.tile([4, R], BF16)
            nc.sync.dma_start(out=sr_s[:4, :], in_=srows[:, :])
            idx_s = cpool.tile([128, 2 * idxcols], I16)
            nc.sync.dma_start(out=idx_s[:], in_=idx_all[:, :])
            dloc_s = cpool.tile([128, nmm], F32)
            nc.sync.dma_start(out=dloc_s[:], in_=dloc_all[:, :])
            nzstrip = cpool.tile([128, TILES * LD], F32)
            nc.sync.dma_start(
                out=nzstrip[:].rearrange("p (t h) -> p t h", h=LD),
                in_=noise_in.ap().rearrange("(t p) h -> p t h", p=128))
            # per-tile 0/1 diagonal leftover-self-loop masks (constant
            # across rounds): dg[t] = ident * dmask[:, t]
            dgs = cpool.tile([128, TILES * 128], BF16)
            for t in range(TILES):
                nc.vector.tensor_scalar(
                    out=dgs[:, t * 128:(t + 1) * 128], in0=ident[:],
                    scalar1=dmask_s[:, t:t + 1], scalar2=None,
                    op0=mybir.AluOpType.mult)

            with tc.tile_pool(name="psum", bufs=4, space="PSUM") as mmpool, \
                 tc.tile_pool(name="hpsum", bufs=4, space="PSUM") as hpool:

                # ---------------- stage 0: X0 = T (Xf A + Xc B) --------------
                with tc.tile_pool(name="strips", bufs=1) as spool:
                    xstrip = [spool.tile([128, R], BF16, name=f"xs{i}")
                              for i in range(2)]
                    with tc.tile_pool(name="s0", bufs=1) as s0pool:
                        xf0_s = s0pool.tile([128, R], BF16)
                        xf1_s = s0pool.tile([128, R], BF16)
                        xc_s = s0pool.tile([128, R], BF16)
                        nc.sync.dma_start(out=xf0_s[:], in_=xfT[0:128, :])
                        nc.sync.dma_start(out=xf1_s[:], in_=xfT[128:256, :])
                        nc.sync.dma_start(out=xc_s[:], in_=xcT[:, :])
                        for t in range(TILES):
                            cs = slice(t * 128, (t + 1) * 128)
                            ps = mmpool.tile([128, HD], F32, name=f"s0ps{t}",
                                             tag="mm")
                            nc.tensor.matmul(ps[:], xf0_s[:, cs], a0_s[:],
                                             start=True, stop=False)
                            nc.tensor.matmul(ps[:], xf1_s[:, cs], a1_s[:],
                                             start=False, stop=False)
                            nc.tensor.matmul(ps[:], xc_s[:, cs], b_s[:],
                                             start=False, stop=True)
                            nc.scalar.activation(
                                out=xstrip[0][:, cs], in_=ps[:],
                                func=mybir.ActivationFunctionType.Copy,
                                scale=dinv_s[:, t:t + 1])
                            if t % TPG == TPG - 1:
                                gg = t // TPG
                                rs = slice(gg * TPG * 128, (t + 1) * 128)
                                nc.sync.dma_start(
                                    out=bounce[0].ap()[rs, :].rearrange(
                                        "(t p) h -> p t h", p=128),
                                    in_=xstrip[0][:, rs].rearrange(
                                        "p (t h) -> p t h", h=HD))

                        if do_cc:
                            nc.gpsimd.collective_compute(
                                "AllGather", mybir.AluOpType.bypass,
                                replica_groups=rg,
                                ins=[bounce[0].ap()], outs=[tabs[0].ap()])

                    # ---------------- 3 sparse rounds ------------------------
                    with tc.tile_pool(name="gpool", bufs=5) as gpool, \
                         tc.tile_pool(name="qpool", bufs=8) as qpool, \
                         tc.tile_pool(name="hd", bufs=4) as hdpool, \
                         tc.tile_pool(name="ostr", bufs=1) as opool:
                        znstrip = opool.tile([128, TILES * LD], F32)
                        mnstrip = opool.tile([128, TILES * LD], F32)
                        lvstrip = opool.tile([128, TILES * LD], F32)
                        for rnd in range(3):
                            xprev = xstrip[rnd % 2]
                            xcur = xstrip[(rnd + 1) % 2]
                            tab = tabs[rnd]
                            tab2 = tab.ap().rearrange(
                                "(r two) h -> r two h", two=2)
                            last = rnd == 2
                            gts = {}   # (par, call) -> gather tile
                            for g in range(NGROUPS):
                                s, e = bounds[g]
                                ncall = e - s
                                for par in (0, 1):
                                    gt = gpool.tile(
                                        [128, ncall, 128], BF16,
                                        name=f"gt{rnd}_{g}_{par}", tag="gath")
                                    ic0 = par * idxcols + \
                                        sum((bounds[j][1] - bounds[j][0]) * 8
                                            for j in range(g))
                                    if do_gather:
                                        nc.gpsimd.dma_gather(
                                            out_ap=gt[:],
                                            in_ap=tab2[:, par, :],
                                            idxs_ap=idx_s[:,
                                                          ic0:ic0 + ncall * 8],
                                            num_idxs=ncall * 128,
                                            num_idxs_reg=ncall * 128,
                                            elem_size=HD,
                                            elem_step=2 * HD,
                                            single_packet=False)
                                    else:
                                        nc.vector.tensor_scalar(
                                            out=gt[:, 0, :], in0=colidx[:],
                                            scalar1=1.0, scalar2=None,
                                            op0=mybir.AluOpType.mult)
                                    gts[(par, g)] = gt
                                for ti in range(TPG):
                                    t = g * TPG + ti
                                    cs = slice(t * 128, (t + 1) * 128)
                                    ps = mmpool.tile([128, HD], F32,
                                                     name=f"ps{rnd}_{t}",
                                                     tag="mm")
                                    tdescs = [d for d in descs if d[2] == t]
                                    if not do_mm:
                                        tdescs = []
                                    qshared = None
                                    for j, (p, c, _, m) in enumerate(tdescs):
                                        cg, coff = call_of_chunk[c]
                                        if do_q or qshared is None:
                                            q = qpool.tile(
                                                [128, 128], BF16,
                                                name=f"q{rnd}_{t}_{j}",
                                                tag="q")
                                            nc.vector.tensor_scalar(
                                                out=q[:], in0=colidx[:],
                                                scalar1=dloc_s[:, m:m + 1],
                                                scalar2=None,
                                                op0=mybir.AluOpType.is_equal)
                                            qshared = q
                                        else:
                                            q = qshared
                                        gsl = gts[(p, cg)][:, coff, :]
                                        if not last:
                                            nc.tensor.matmul(
                                                ps[:], q[:], gsl,
                                                start=(j == 0), stop=False)
                                        else:
                                            nc.tensor.matmul(
                                                ps[:], gsl, q[:],
                                                start=(j == 0), stop=False)
                                    dg = dgs[:, t * 128:(t + 1) * 128]
                                    if not last:
                                        nc.tensor.matmul(
                                            ps[:], dg, xprev[:, cs],
                                            start=not tdescs, stop=True)
                                        nc.scalar.activation(
                                            out=xcur[:, cs], in_=ps[:],
                                            func=mybir.ActivationFunctionType.Copy,
                                            scale=dinv2_s[:, t:t + 1])
                                        if ti == TPG - 1:
                                            rs = slice(g * TPG * 128,
                                                       (t + 1) * 128)
                                            nc.sync.dma_start(
                                                out=bounce[rnd + 1].ap()[
                                                    rs, :].rearrange(
                                                    "(t p) h -> p t h",
                                                    p=128),
                                                in_=xcur[:, rs].rearrange(
                                                    "p (t h) -> p t h",
                                                    h=HD))
                                    else:
                                        nc.tensor.matmul(
                                            ps[:], xprev[:, cs], dg,
                                            start=not tdescs, stop=True)
                                        # ps is rawV^T [feat, node]
                                        vT = hdpool.tile([128, HD], BF16,
                                                         name=f"vT{t}",
                                                         tag="vT")
                                        nc.vector.tensor_copy(out=vT[:],
                                                              in_=ps[:])
                                        os = slice(t * LD, (t + 1) * LD)
                                        mps = hpool.tile([128, LD], F32,
                                                         name=f"mps{t}",
                                                         tag="hp")
                                        nc.tensor.matmul(mps[:], vT[:],
                                                         wm_s[:],
                                                         start=True,
                                                         stop=False)
                                        nc.tensor.matmul(mps[:],
                                                         sr_s[:3, cs],
                                                         cm_s[:3, :],
                                                         start=False,
                                                         stop=True)
                                        lps = hpool.tile([128, LD], F32,
                                                         name=f"lps{t}",
                                                         tag="hp")
                                        nc.tensor.matmul(lps[:], vT[:],
                                                         wv_s[:],
                                                         start=True,
                                                         stop=False)
                                        nc.tensor.matmul(lps[:],
                                                         sr_s[:3, cs],
                                                         cv_s[:3, :],
                                                         start=False,
                                                         stop=True)
                                        nc.vector.tensor_scalar(
                                            out=mnstrip[:, os], in0=mps[:],
                                            scalar1=dinv_s[:, t:t + 1],
                                            scalar2=None,
                                            op0=mybir.AluOpType.mult)
                                        nc.vector.tensor_scalar(
                                            out=lvstrip[:, os], in0=lps[:],
                                            scalar1=dinv_s[:, t:t + 1],
                                            scalar2=None,
                                            op0=mybir.AluOpType.mult)
                                        ex = hdpool.tile([128, LD], F32,
                                                         name=f"ex{t}",
                                                         tag="ex")
                                        nc.scalar.activation(
                                            out=ex[:], in_=lps[:],
                                            func=mybir.ActivationFunctionType.Exp,
                                            scale=dinvh_s[:, t:t + 1])
                                        zt = hdpool.tile([128, LD], F32,
                                                         name=f"zt{t}",
                                                         tag="zt")
                                        nc.vector.tensor_tensor(
                                            out=zt[:], in0=nzstrip[:, os],
                                            in1=ex[:],
                                            op=mybir.AluOpType.mult)
                                        nc.vector.tensor_tensor(
                                            out=znstrip[:, os], in0=zt[:],
                                            in1=mnstrip[:, os],
                                            op=mybir.AluOpType.add)
                            if not last:
                                if do_cc:
                                    nc.gpsimd.collective_compute(
                                        "AllGather", mybir.AluOpType.bypass,
                                        replica_groups=rg,
                                        ins=[bounce[rnd + 1].ap()],
                                        outs=[tabs[rnd + 1].ap()])
                        nc.sync.dma_start(
                            out=z_out.ap().rearrange("(t p) h -> p t h",
                                                     p=128),
                            in_=znstrip[:].rearrange("p (t h) -> p t h",
                                                     h=LD))
                        nc.sync.dma_start(
                            out=mean_out.ap().rearrange("(t p) h -> p t h",
                                                        p=128),
                            in_=mnstrip[:].rearrange("p (t h) -> p t h",
                                                     h=LD))
                        nc.sync.dma_start(
                            out=logvar_out.ap().rearrange("(t p) h -> p t h",
                                                          p=128),
                            in_=lvstrip[:].rearrange("p (t h) -> p t h",
                                                     h=LD))
    nc.finalize()
    return nc


# --------------------------------------------------------------------------
# Host-side preprocessing
# --------------------------------------------------------------------------
def preprocess(feature, condition, edge_index, noise,
               W1, b1, W2, b2, W3, b3, Wm, bm, Wv, bv):
    feature = np.asarray(feature, np.float32)
    condition = np.asarray(condition, np.float32)
    noise = np.asarray(noise, np.float32)
    ei = np.asarray(edge_index).astype(np.int64)
    W1 = np.asarray(W1, np.float32); b1 = np.asarray(b1, np.float32)
    W2 = np.asarray(W2, np.float32); b2 = np.asarray(b2, np.float32)
    W3 = np.asarray(W3, np.float32); b3 = np.asarray(b3, np.float32)
    Wm = np.asarray(Wm, np.float32); bm = np.asarray(bm, np.float32)
    Wv = np.asarray(Wv, np.float32); bv = np.asarray(bv, np.float32)

    src, dst = ei[0], ei[1]
    loop = np.arange(N, dtype=np.int64)
    deg = (np.bincount(np.concatenate([dst, loop]), minlength=N)
           .astype(np.float64))
    dinv = 1.0 / np.sqrt(deg)
    asrc = np.concatenate([src, loop])
    adst = np.concatenate([dst, loop])
    w = dinv[asrc] * dinv[adst]
    s1 = np.bincount(adst, weights=w, minlength=N)
    s2 = np.bincount(adst, weights=w * s1[asrc], minlength=N)
    dinv32 = dinv.astype(np.float32)

    W3a, W3b = W3[:HD], W3[HD:]
    A_w = (W1 @ W3a).astype(np.float32)
    B_w = (W2 @ W3b).astype(np.float32)
    c1 = b1 @ W3a + b2 @ W3b
    Cm = np.zeros((4, LD), np.float32)
    Cm[:3] = np.stack([c1 @ Wm, b3 @ Wm, bm])
    Cv = np.zeros((4, LD), np.float32)
    Cv[:3] = np.stack([c1 @ Wv, b3 @ Wv, bv])

    node = np.arange(N, dtype=np.int64)
    pos_of_node = (node // SHARD) * R + (node % SHARD)
    pos_src = pos_of_node[src]
    core = dst // SHARD
    d_loc = dst - core * SHARD
    tl = d_loc // 128
    dloc = d_loc % 128
    par = (pos_src & 1).astype(np.int64)
    idx16 = (pos_src >> 1).astype(np.int64)

    # counts per (core, tile, parity) -> CAP
    gid = (core * TILES + tl) * 2 + par
    counts = np.bincount(gid, minlength=CORES * TILES * 2)
    cap = int(counts.max())

    descs, nmm = _mm_descs(cap)
    nchk, bounds = _call_bounds(cap)
    nslot = nchk * 128

    # order edges by (core, tile, parity), dloc ascending within the group
    order = np.lexsort((dloc, gid))
    gs = gid[order]
    starts = np.concatenate([[0], np.cumsum(counts)[:-1]])
    within = np.arange(len(gs)) - np.repeat(starts, counts)

    ocore = core[order]
    opar = par[order]
    otl = tl[order]
    # slot index within the (core, parity) block
    slot = otl * cap + within

    bf = np.float32  # host arrays later cast

    in_maps = []
    for k in range(CORES):
        rows = slice(k * SHARD, (k + 1) * SHARD)
        idx_p = np.zeros((2, nslot), np.int16)
        dl_p = np.full((2, nslot), -1.0, np.float32)
        fill_n = np.zeros((2, TILES), np.int64)
        for p in (0, 1):
            m = (ocore == k) & (opar == p)
            idx_p[p][slot[m]] = idx16[order][m].astype(np.int16)
            dl_p[p][slot[m]] = dloc[order][m].astype(np.float32)

        # self-loop filler: node i (tile t, row parity q) contributes X[i]
        # to psum[i]; append into slack slots of segment (t, q).
        dmask = np.zeros((TILES, 128), np.float32)
        cnt_k = counts.reshape(CORES, TILES, 2)[k]
        for t in range(TILES):
            nreal = min(128, SHARD - t * 128)
            i_local = t * 128 + np.arange(nreal)
            pos_i = k * R + i_local
            q_i = pos_i & 1
            for p in (0, 1):
                cand = i_local[q_i == p]
                space = cap - cnt_k[t, p]
                use = cand[:space]
                rest = cand[space:]
                base = t * cap + cnt_k[t, p]
                idx_p[p][base:base + len(use)] = (
                    (k * R + use) >> 1).astype(np.int16)
                dl_p[p][base:base + len(use)] = (use - t * 128).astype(
                    np.float32)
                dmask[t][rest - t * 128] = 1.0

        # wrap idx lists per call: [16, n/16] replicated to 128 partitions
        idx_cols = []
        for p in (0, 1):
            for s, e in bounds:
                iv = idx_p[p][s * 128:e * 128]
                ic = iv.reshape(-1, 16).T          # [16, n/16]
                idx_cols.append(ic)
        # order in SBUF: parity-major (par * idxcols + call offset)
        ic_all = np.concatenate(idx_cols, axis=1)
        idx_arr = np.tile(ic_all, (8, 1))

        # dloc columns per matmul descriptor
        dl_arr = np.full((128, nmm), -1.0, np.float32)
        for p, c, t, mcol in descs:
            seg = dl_p[p][c * 128:(c + 1) * 128]
            # the chunk may contain other tiles' edges -> keep -1 there
            s0, s1_ = t * cap, (t + 1) * cap
            pos = np.arange(c * 128, (c + 1) * 128)
            ok = (pos >= s0) & (pos < s1_) & (seg >= 0)
            col = np.where(ok, seg, -1.0)
            dl_arr[:, mcol] = col

        xfTb = np.zeros((FD, R), bf)
        xfTb[:, :SHARD] = feature[rows].T
        xcTb = np.zeros((CD, R), bf)
        xcTb[:, :SHARD] = condition[rows].T
        nz = np.zeros((R, LD), np.float32)
        nz[:SHARD] = noise[rows]
        dv = np.zeros((TILES, 128), np.float32)
        dv.reshape(-1)[:SHARD] = dinv32[rows]
        sr = np.zeros((4, R), np.float32)
        di = dinv[rows]
        sr[0, :SHARD] = (s2[rows] / di).astype(np.float32)
        sr[1, :SHARD] = (s1[rows] / di).astype(np.float32)
        sr[2, :SHARD] = (1.0 / di).astype(np.float32)

        in_maps.append({
            "xfT": xfTb, "xcT": xcTb, "noise_in": nz,
            "aw": A_w, "bw": B_w, "wm": Wm, "wv": Wv, "cm": Cm, "cv": Cv,
            "dinv_p": np.ascontiguousarray(dv.T),
            "dinv2_p": np.ascontiguousarray((dv ** 2).T),
            "dinvh_p": np.ascontiguousarray(0.5 * dv.T),
            "dmask_p": np.ascontiguousarray(dmask.T),
            "srows": sr,
            "idx_all": np.ascontiguousarray(idx_arr),
            "dloc_all": np.ascontiguousarray(dl_arr),
        })
    return cap, in_maps


def _cast_maps(in_maps, nc):
    """Cast host fp32 arrays to the program's declared dtypes (bf16)."""
    import ml_dtypes
    bf16_names = {"xfT", "xcT", "aw", "bw", "wm", "wv", "cm", "cv",
                  "srows"}
    out = []
    for m in in_maps:
        d = dict(m)
        for n in bf16_names:
            d[n] = m[n].astype(ml_dtypes.bfloat16)
        out.append(d)
    return out


def kernel(feature, condition, edge_index, noise,
           W1, b1, W2, b2, W3, b3, Wm, bm, Wv, bv, _trace=False):
    cap, in_maps = preprocess(feature, condition, edge_index, noise,
                              W1, b1, W2, b2, W3, b3, Wm, bm, Wv, bv)
    if cap not in _prog_cache:
        _prog_cache[cap] = build_program(cap)
    nc = _prog_cache[cap]
    in_maps = _cast_maps(in_maps, nc)
    res = run_bass_kernel_spmd(nc, in_maps, list(range(CORES)), trace=_trace)
    z = np.concatenate([res.results[k]["z_out"][:SHARD] for k in range(CORES)])
    mean = np.concatenate(
        [res.results[k]["mean_out"][:SHARD] for k in range(CORES)])
    logvar = np.concatenate(
        [res.results[k]["logvar_out"][:SHARD] for k in range(CORES)])
    return (z, mean, logvar)
